# revision 1
# baseline (speedup 1.0000x reference)
"""CropToBBox (crop_and_resize to 224x224 with bbox preprocessing) on 8 trn2 cores.

Strategy: data-parallel over N=64 images (8 per core). Bilinear crop+resize is
separable: out_c = Ry @ I_c @ Rx^T per channel, where Ry/Rx are [224, 512]
interpolation matrices with triangle (hat) weights relu(1 - |ys_i - h|),
masked to zero for out-of-range sample positions.

Device pipeline per image:
  stage 1: V^T_c[w, i] = sum_h I[h, w, c] * RyT[h, i]   (lhsT = image slice)
  stage 2: O_c[i, j]   = sum_w V^T_c[w, i] * RxT[w, j]
Both as exact float32 matmuls. Ry/Rx built
on-device by ScalarE from host-computed sample coordinates (2 ops per
128-chunk: u = Abs(ys - w), then Relu(-u + 1)).

Host computes ys/xs [224] per image in fp32 replicating the reference bbox
math bit-exactly; invalid (out-of-range) positions are set to -1e5 so all
triangle weights vanish (matches the reference's zero-fill masking).
"""

import numpy as np

N_FULL = 64
H = W = 512
C = 3
CH = CW = 224
NPAD = 224  # == CH/CW; fp32 matmuls have no min-N constraint
N_CORES = 8
PER_CORE = N_FULL // N_CORES
FACTOR = 1.2

_CACHE = {}


def _host_coords(threshold, bboxes):
    """Replicate process_bbox + crop_and_resize coordinate math in fp32."""
    f = np.float32
    th = np.asarray(threshold, f)
    bb = np.asarray(bboxes, f)
    default = np.array([0.0, 1.0, 0.0, 1.0], f)
    filt = np.where(th < f(0.5), default, bb).astype(f)
    x1, y1, x2, y2 = filt[:, 0], filt[:, 1], filt[:, 2], filt[:, 3]

    def resize_side(small, large):
        side = (large - small).astype(f)
        new_side = (side * f(FACTOR)).astype(f)
        center = ((small + large) / f(2)).astype(f)
        half = (new_side / f(2)).astype(f)
        new_min = np.clip((center - half).astype(f), f(0), f(1)).astype(f)
        new_max = np.clip((center + half).astype(f), f(0), f(1)).astype(f)
        return new_min, new_max

    nx1, nx2 = resize_side(x1, x2)
    ny1, ny2 = resize_side(y1, y2)
    # reference: boxes = stack([nx1, ny1, nx2, ny2]); crop uses [y1,x1,y2,x2]
    by1, bx1, by2, bx2 = nx1, ny1, nx2, ny2

    idx = np.arange(CH, dtype=f)
    ys = (by1[:, None] * f(H - 1)).astype(f) + (
        idx[None, :] * (((by2 - by1) * f(H - 1)).astype(f) / f(CH - 1)).astype(f)[:, None]
    ).astype(f)
    ys = ys.astype(f)
    xs = (bx1[:, None] * f(W - 1)).astype(f) + (
        idx[None, :] * (((bx2 - bx1) * f(W - 1)).astype(f) / f(CW - 1)).astype(f)[:, None]
    ).astype(f)
    xs = xs.astype(f)

    BAD = f(-1e5)
    ys = np.where((ys >= f(0)) & (ys <= f(H - 1)), ys, BAD).astype(f)
    xs = np.where((xs >= f(0)) & (xs <= f(W - 1)), xs, BAD).astype(f)

    ys_pad = np.full((N_FULL, NPAD), BAD, f)
    xs_pad = np.full((N_FULL, NPAD), BAD, f)
    ys_pad[:, :CH] = ys
    xs_pad[:, :CW] = xs
    return ys_pad, xs_pad


def _build_nc():
    from concourse import bacc, tile
    import concourse.mybir as mybir

    dt = mybir.dt
    F32 = dt.float32
    F32R = dt.float32r
    AF = mybir.ActivationFunctionType

    # Bacc (not raw Bass): its compile pipeline splits semaphore waits into
    # event-semaphore instructions, satisfying the TRN2 1-wait-per-instruction
    # ISA constraint that walrus codegen enforces.
    nc = bacc.Bacc(None, target_bir_lowering=False)
    images_d = nc.declare_dram_parameter("images", [PER_CORE, H, W, C], F32, isOutput=False)
    ys_d = nc.declare_dram_parameter("ys", [PER_CORE, NPAD], F32, isOutput=False)
    xs_d = nc.declare_dram_parameter("xs", [PER_CORE, NPAD], F32, isOutput=False)
    wneg_d = nc.declare_dram_parameter("wneg", [128, 4], F32, isOutput=False)
    out_d = nc.declare_dram_parameter("out", [PER_CORE, CH, CW, C], F32, isOutput=True)

    KH = H // 128  # 4 h-chunks
    KW = W // 128  # 4 w-chunks
    IC = 2         # i-chunks of 112
    ICH = CH // IC

    with tile.TileContext(nc) as tc:
        with (
            tc.tile_pool(name="const", bufs=1) as cpool,
            tc.tile_pool(name="img", bufs=3) as ipool,
            tc.tile_pool(name="wts", bufs=8) as wpool,
            tc.tile_pool(name="tmp", bufs=3) as tpool,
            tc.tile_pool(name="vt", bufs=24) as vpool,
            tc.tile_pool(name="outsb", bufs=3) as opool,
            tc.tile_pool(name="psv", bufs=4, space="PSUM") as psv_pool,
            tc.tile_pool(name="pso", bufs=3, space="PSUM") as pso_pool,
            tc.tile_pool(name="bc", bufs=2) as bcpool,
        ):
            # issue image 0's load first: it is the longest pole in the
            # pipeline fill; split it so the first h-chunks land early and
            # stage-1 matmuls can start while the rest streams in
            img0 = ipool.tile([128, KH, W, C], F32, name="img4_0", tag="img4")
            for lo, hi in ((0, 1), (1, 2), (2, 4)):
                nc.sync.dma_start(
                    out=img0[:, lo:hi],
                    in_=images_d[0, 128 * lo:128 * hi].rearrange(
                        "(kh p) w c -> p kh w c", p=128),
                )

            wneg = cpool.tile([128, 4], F32)
            nc.scalar.dma_start(out=wneg[:], in_=wneg_d[:])

            for n in range(PER_CORE):
                if n == 0:
                    img4 = img0
                else:
                    img4 = ipool.tile([128, KH, W, C], F32, name=f"img4_{n}", tag="img4")
                    nc.sync.dma_start(
                        out=img4[:],
                        in_=images_d[n].rearrange("(kh p) w c -> p kh w c", p=128),
                    )
                img = [img4[:, k] for k in range(KH)]

                # per-image coordinate rows (ScalarE HWDGE queue, off the
                # SP image-DMA path), then broadcast to 128 partitions on
                # GpSimd (otherwise idle)
                ysr = bcpool.tile([1, NPAD], F32, name=f"ysr_{n}", tag="ysr")
                nc.scalar.dma_start(out=ysr[:], in_=ys_d[n].unsqueeze(0))
                xsr = bcpool.tile([1, NPAD], F32, name=f"xsr_{n}", tag="xsr")
                nc.scalar.dma_start(out=xsr[:], in_=xs_d[n].unsqueeze(0))
                ysb = bcpool.tile([128, NPAD], F32, tag="ysb")
                nc.gpsimd.partition_broadcast(ysb[:], ysr[:])
                xsb = bcpool.tile([128, NPAD], F32, tag="xsb")
                nc.gpsimd.partition_broadcast(xsb[:], xsr[:])

                # interpolation weight chunks: [128, NPAD] per 128-row window
                ryt = []
                rxt = []
                for k in range(KH):
                    u = tpool.tile([128, NPAD], F32)
                    nc.scalar.activation(u[:], ysb[:], AF.Abs, bias=wneg[:, k:k + 1], scale=1.0)
                    r = wpool.tile([128, NPAD], F32, tag="ryt")
                    nc.scalar.activation(r[:], u[:], AF.Relu, bias=1.0, scale=-1.0)
                    ryt.append(r)
                for k in range(KW):
                    u2 = tpool.tile([128, NPAD], F32, tag="u")
                    nc.scalar.activation(u2[:], xsb[:], AF.Abs, bias=wneg[:, k:k + 1], scale=1.0)
                    r = wpool.tile([128, NPAD], F32, tag="rxt")
                    nc.scalar.activation(r[:], u2[:], AF.Relu, bias=1.0, scale=-1.0)
                    rxt.append(r)

                # stage 1: V^T_c[w_chunk][p=w, i] = sum_h I[h, w, c] RyT[h, i]
                vt = {}
                for ci in range(C):
                    for wk in range(KW):
                        pv = psv_pool.tile([128, NPAD], F32)
                        for kh in range(KH):
                            nc.tensor.matmul(
                                pv[:],
                                img[kh][:, wk * 128:(wk + 1) * 128, ci],
                                ryt[kh][:],
                                start=(kh == 0),
                                stop=(kh == KH - 1),
                            )
                        v = vpool.tile([128, CH], F32, tag="vt")
                        nc.vector.tensor_copy(v[:], pv[:, :CH])
                        vt[(ci, wk)] = v

                # stage 2 + channel interleave + store
                for ic in range(IC):
                    osb = opool.tile([ICH, CW, C], F32)
                    for ci in range(C):
                        po = pso_pool.tile([ICH, NPAD], F32)
                        for wk in range(KW):
                            nc.tensor.matmul(
                                po[:],
                                vt[(ci, wk)][:, ic * ICH:(ic + 1) * ICH],
                                rxt[wk][:],
                                start=(wk == 0),
                                stop=(wk == KW - 1),
                            )
                        nc.vector.tensor_copy(osb[:, :, ci], po[:, :CW])
                    nc.sync.dma_start(
                        out=out_d[n, ic * ICH:(ic + 1) * ICH], in_=osb[:]
                    )
    nc.finalize()
    return nc


def _get_nc():
    if "nc" not in _CACHE:
        _CACHE["nc"] = _build_nc()
    return _CACHE["nc"]


def _wneg_const():
    p = np.arange(128, dtype=np.float32)
    return np.stack([-(128.0 * k + p) for k in range(4)], axis=1).astype(np.float32)


def _ensure_device_platform():
    """If the process pinned jax to cpu (e.g. JAX_PLATFORMS=cpu), re-resolve
    backends so the 8 axon/neuron devices are visible for the PJRT run."""
    import jax
    try:
        if len([d for d in jax.devices() if d.platform != "cpu"]) >= N_CORES:
            return
    except Exception:
        pass
    import os
    os.environ.pop("JAX_PLATFORMS", None)
    try:
        jax.config.update("jax_platforms", None)
    except Exception:
        pass
    for clear in ("clear_backends",):
        try:
            getattr(jax, clear)()
            break
        except Exception:
            pass


def kernel(threshold, bboxes, images):
    from concourse.bass_utils import run_bass_kernel_spmd

    _ensure_device_platform()

    ys_pad, xs_pad = _host_coords(threshold, bboxes)
    images = np.ascontiguousarray(np.asarray(images, np.float32))
    wneg = _wneg_const()

    nc = _get_nc()
    in_maps = []
    for core in range(N_CORES):
        sl = slice(core * PER_CORE, (core + 1) * PER_CORE)
        in_maps.append({
            "images": images[sl],
            "ys": np.ascontiguousarray(ys_pad[sl]),
            "xs": np.ascontiguousarray(xs_pad[sl]),
            "wneg": wneg,
        })
    import os
    trace = bool(os.environ.get("CROP_TRACE"))
    if trace:
        try:
            import antenv.axon_hooks  # noqa: F401
        except ImportError:
            trace = False
    res = run_bass_kernel_spmd(nc, in_maps, list(range(N_CORES)), trace=trace)
    _CACHE["last_res"] = res
    out = np.concatenate([res.results[i]["out"] for i in range(N_CORES)], axis=0)
    return out.astype(np.float32)



# revision 3
# speedup vs baseline: 2.1902x; 2.1902x over previous
"""CropToBBox (crop_and_resize to 224x224 with bbox preprocessing) on 8 trn2 cores.

Strategy: data-parallel over N=64 images (8 per core). Bilinear crop+resize is
separable: out_c = Ry @ I_c @ Rx^T per channel, where Ry/Rx are [224, 512]
interpolation matrices with triangle (hat) weights relu(1 - |ys_i - h|),
masked to zero for out-of-range sample positions.

Device pipeline per image:
  stage 1: V^T_c[w, i] = sum_h I[h, w, c] * RyT[h, i]   (lhsT = image slice)
  stage 2: O_c[i, j]   = sum_w V^T_c[w, i] * RxT[w, j]
Both as exact float32 matmuls. Ry/Rx built
on-device by ScalarE from host-computed sample coordinates (2 ops per
128-chunk: u = Abs(ys - w), then Relu(-u + 1)).

Host computes ys/xs [224] per image in fp32 replicating the reference bbox
math bit-exactly; invalid (out-of-range) positions are set to -1e5 so all
triangle weights vanish (matches the reference's zero-fill masking).
"""

import numpy as np

N_FULL = 64
H = W = 512
C = 3
CH = CW = 224
NPAD = 256  # f32r matmuls need out free >= 256 for 1 cycle/row
N_CORES = 8
PER_CORE = N_FULL // N_CORES
FACTOR = 1.2

_CACHE = {}


def _host_coords(threshold, bboxes):
    """Replicate process_bbox + crop_and_resize coordinate math in fp32."""
    f = np.float32
    th = np.asarray(threshold, f)
    bb = np.asarray(bboxes, f)
    default = np.array([0.0, 1.0, 0.0, 1.0], f)
    filt = np.where(th < f(0.5), default, bb).astype(f)
    x1, y1, x2, y2 = filt[:, 0], filt[:, 1], filt[:, 2], filt[:, 3]

    def resize_side(small, large):
        side = (large - small).astype(f)
        new_side = (side * f(FACTOR)).astype(f)
        center = ((small + large) / f(2)).astype(f)
        half = (new_side / f(2)).astype(f)
        new_min = np.clip((center - half).astype(f), f(0), f(1)).astype(f)
        new_max = np.clip((center + half).astype(f), f(0), f(1)).astype(f)
        return new_min, new_max

    nx1, nx2 = resize_side(x1, x2)
    ny1, ny2 = resize_side(y1, y2)
    # reference: boxes = stack([nx1, ny1, nx2, ny2]); crop uses [y1,x1,y2,x2]
    by1, bx1, by2, bx2 = nx1, ny1, nx2, ny2

    idx = np.arange(CH, dtype=f)
    ys = (by1[:, None] * f(H - 1)).astype(f) + (
        idx[None, :] * (((by2 - by1) * f(H - 1)).astype(f) / f(CH - 1)).astype(f)[:, None]
    ).astype(f)
    ys = ys.astype(f)
    xs = (bx1[:, None] * f(W - 1)).astype(f) + (
        idx[None, :] * (((bx2 - bx1) * f(W - 1)).astype(f) / f(CW - 1)).astype(f)[:, None]
    ).astype(f)
    xs = xs.astype(f)

    BAD = f(-1e5)
    ys = np.where((ys >= f(0)) & (ys <= f(H - 1)), ys, BAD).astype(f)
    xs = np.where((xs >= f(0)) & (xs <= f(W - 1)), xs, BAD).astype(f)

    ys_pad = np.full((N_FULL, NPAD), BAD, f)
    xs_pad = np.full((N_FULL, NPAD), BAD, f)
    ys_pad[:, :CH] = ys
    xs_pad[:, :CW] = xs
    return ys_pad, xs_pad


def _build_nc():
    from concourse import bacc, tile
    import concourse.mybir as mybir

    dt = mybir.dt
    F32 = dt.float32
    F32R = dt.float32r
    AF = mybir.ActivationFunctionType

    # Bacc (not raw Bass): its compile pipeline splits semaphore waits into
    # event-semaphore instructions, satisfying the TRN2 1-wait-per-instruction
    # ISA constraint that walrus codegen enforces.
    nc = bacc.Bacc(None, target_bir_lowering=False)
    images_d = nc.declare_dram_parameter("images", [PER_CORE, H, W, C], F32R, isOutput=False)
    ys_d = nc.declare_dram_parameter("ys", [PER_CORE, NPAD], F32, isOutput=False)
    xs_d = nc.declare_dram_parameter("xs", [PER_CORE, NPAD], F32, isOutput=False)
    wneg_d = nc.declare_dram_parameter("wneg", [128, 4], F32, isOutput=False)
    out_d = nc.declare_dram_parameter("out", [PER_CORE, CH, CW, C], F32, isOutput=True)

    KH = H // 128  # 4 h-chunks
    KW = W // 128  # 4 w-chunks
    IC = 2         # i-chunks of 112
    ICH = CH // IC

    with tile.TileContext(nc) as tc:
        with (
            tc.tile_pool(name="const", bufs=1) as cpool,
            tc.tile_pool(name="img", bufs=3) as ipool,
            tc.tile_pool(name="wts", bufs=8) as wpool,
            tc.tile_pool(name="tmp", bufs=3) as tpool,
            tc.tile_pool(name="vt", bufs=24) as vpool,
            tc.tile_pool(name="outsb", bufs=3) as opool,
            tc.tile_pool(name="psv", bufs=4, space="PSUM") as psv_pool,
            tc.tile_pool(name="pso", bufs=3, space="PSUM") as pso_pool,
            tc.tile_pool(name="bc", bufs=2) as bcpool,
        ):
            # issue image 0's load first: it is the longest pole in the
            # pipeline fill; split it so the first h-chunks land early and
            # stage-1 matmuls can start while the rest streams in
            img0 = ipool.tile([128, KH, W, C], F32R, name="img4_0", tag="img4")
            for lo, hi in ((0, 1), (1, 2), (2, 4)):
                nc.sync.dma_start(
                    out=img0[:, lo:hi],
                    in_=images_d[0, 128 * lo:128 * hi].rearrange(
                        "(kh p) w c -> p kh w c", p=128),
                )

            wneg = cpool.tile([128, 4], F32)
            nc.scalar.dma_start(out=wneg[:], in_=wneg_d[:])

            for n in range(PER_CORE):
                if n == 0:
                    img4 = img0
                else:
                    img4 = ipool.tile([128, KH, W, C], F32R, name=f"img4_{n}", tag="img4")
                    nc.sync.dma_start(
                        out=img4[:],
                        in_=images_d[n].rearrange("(kh p) w c -> p kh w c", p=128),
                    )
                img = [img4[:, k] for k in range(KH)]

                # per-image coordinate rows (ScalarE HWDGE queue, off the
                # SP image-DMA path), then broadcast to 128 partitions on
                # GpSimd (otherwise idle)
                ysr = bcpool.tile([1, NPAD], F32, name=f"ysr_{n}", tag="ysr")
                nc.scalar.dma_start(out=ysr[:], in_=ys_d[n].unsqueeze(0))
                xsr = bcpool.tile([1, NPAD], F32, name=f"xsr_{n}", tag="xsr")
                nc.scalar.dma_start(out=xsr[:], in_=xs_d[n].unsqueeze(0))
                ysb = bcpool.tile([128, NPAD], F32, tag="ysb")
                nc.gpsimd.partition_broadcast(ysb[:], ysr[:])
                xsb = bcpool.tile([128, NPAD], F32, tag="xsb")
                nc.gpsimd.partition_broadcast(xsb[:], xsr[:])

                # interpolation weight chunks: [128, NPAD] per 128-row window
                ryt = []
                rxt = []
                for k in range(KH):
                    u = tpool.tile([128, NPAD], F32)
                    nc.scalar.activation(u[:], ysb[:], AF.Abs, bias=wneg[:, k:k + 1], scale=1.0)
                    r = wpool.tile([128, NPAD], F32R, tag="ryt")
                    nc.scalar.activation(r[:], u[:], AF.Relu, bias=1.0, scale=-1.0)
                    ryt.append(r)
                for k in range(KW):
                    u2 = tpool.tile([128, NPAD], F32, tag="u")
                    nc.scalar.activation(u2[:], xsb[:], AF.Abs, bias=wneg[:, k:k + 1], scale=1.0)
                    r = wpool.tile([128, NPAD], F32R, tag="rxt")
                    nc.scalar.activation(r[:], u2[:], AF.Relu, bias=1.0, scale=-1.0)
                    rxt.append(r)

                # stage 1: V^T_c[w_chunk][p=w, i] = sum_h I[h, w, c] RyT[h, i]
                vt = {}
                for ci in range(C):
                    for wk in range(KW):
                        pv = psv_pool.tile([128, NPAD], F32)
                        for kh in range(KH):
                            nc.tensor.matmul(
                                pv[:],
                                img[kh][:, wk * 128:(wk + 1) * 128, ci],
                                ryt[kh][:],
                                start=(kh == 0),
                                stop=(kh == KH - 1),
                            )
                        v = vpool.tile([128, CH], F32R, tag="vt")
                        nc.vector.tensor_copy(v[:], pv[:, :CH])
                        vt[(ci, wk)] = v

                # stage 2 + channel interleave + store
                for ic in range(IC):
                    osb = opool.tile([ICH, CW, C], F32)
                    for ci in range(C):
                        po = pso_pool.tile([ICH, NPAD], F32)
                        for wk in range(KW):
                            nc.tensor.matmul(
                                po[:],
                                vt[(ci, wk)][:, ic * ICH:(ic + 1) * ICH],
                                rxt[wk][:],
                                start=(wk == 0),
                                stop=(wk == KW - 1),
                            )
                        nc.vector.tensor_copy(osb[:, :, ci], po[:, :CW])
                    nc.sync.dma_start(
                        out=out_d[n, ic * ICH:(ic + 1) * ICH], in_=osb[:]
                    )
    nc.finalize()
    return nc


def _get_nc():
    if "nc" not in _CACHE:
        _CACHE["nc"] = _build_nc()
    return _CACHE["nc"]


def _wneg_const():
    p = np.arange(128, dtype=np.float32)
    return np.stack([-(128.0 * k + p) for k in range(4)], axis=1).astype(np.float32)


def _ensure_device_platform():
    """If the process pinned jax to cpu (e.g. JAX_PLATFORMS=cpu), re-resolve
    backends so the 8 axon/neuron devices are visible for the PJRT run."""
    import jax
    try:
        if len([d for d in jax.devices() if d.platform != "cpu"]) >= N_CORES:
            return
    except Exception:
        pass
    import os
    os.environ.pop("JAX_PLATFORMS", None)
    try:
        jax.config.update("jax_platforms", None)
    except Exception:
        pass
    for clear in ("clear_backends",):
        try:
            getattr(jax, clear)()
            break
        except Exception:
            pass


def kernel(threshold, bboxes, images):
    from concourse.bass_utils import run_bass_kernel_spmd

    _ensure_device_platform()

    ys_pad, xs_pad = _host_coords(threshold, bboxes)
    images = np.ascontiguousarray(np.asarray(images, np.float32))
    wneg = _wneg_const()

    nc = _get_nc()
    in_maps = []
    for core in range(N_CORES):
        sl = slice(core * PER_CORE, (core + 1) * PER_CORE)
        in_maps.append({
            "images": images[sl],
            "ys": np.ascontiguousarray(ys_pad[sl]),
            "xs": np.ascontiguousarray(xs_pad[sl]),
            "wneg": wneg,
        })
    import os
    trace = bool(os.environ.get("CROP_TRACE"))
    if trace:
        try:
            import antenv.axon_hooks  # noqa: F401
        except ImportError:
            trace = False
    res = run_bass_kernel_spmd(nc, in_maps, list(range(N_CORES)), trace=trace)
    _CACHE["last_res"] = res
    out = np.concatenate([res.results[i]["out"] for i in range(N_CORES)], axis=0)
    return out.astype(np.float32)



# revision 12
# speedup vs baseline: 9.3640x; 4.2754x over previous
"""CropToBBox (crop_and_resize to 224x224 with bbox preprocessing) on 8 trn2 cores.

Strategy: data-parallel over N=64 images, 8 per core, one uniform SPMD program.

Separable bilinear resize per channel: out_c = Ry @ I_c @ Rx^T where Ry/Rx are
[224, rows]/[224, cols] hat-weight matrices. Weights are computed on the host
(exact fp32 bbox math, replicating the reference bit-for-bit) and shipped as
bf16 tensors; images are shipped as bf16 crops of only the rows/cols the box
actually reads.

Images are sorted by crop-window shape and dealt into 8 "slots" so that the
single SPMD program (shapes, matmul intervals) is valid for every core: slot
dims are the max over the 8 members, and matmul column intervals are the union
of the members' nonzero-weight intervals per 128-row chunk. Outside those
intervals all hat weights vanish, so restricted matmuls are exact.

Device pipeline per image slot:
  stage 1: V^T[w, i] += I[h_chunk, w, c] @ Ry^T[h_chunk, i-interval]
  stage 2: O[i, j]   += V^T[w_chunk, i] @ Rx^T[w_chunk, j-interval]
PSUM is evacuated by Pool/DVE/Act engine copies (greedy-balanced), stores are
bf16 and DMAs are greedy-balanced across the SP/Act/Pool DMA queues.
"""

import numpy as np

N_FULL = 64
H = W = 512
C = 3
CH = CW = 224
N_CORES = 8
PER_CORE = N_FULL // N_CORES
FACTOR = 1.2
IPAD = 256  # weight row padding so DMA elem >= 512B
BAD = np.float32(-1e5)
EPS = 1e-3

_CACHE = {}


# ---------------------------------------------------------------- host math

def _host_coords(threshold, bboxes):
    """Replicate process_bbox + crop_and_resize coordinate math in fp32.
    Returns ys, xs [N, 224] with invalid positions set to BAD."""
    f = np.float32
    th = np.asarray(threshold, f)
    bb = np.asarray(bboxes, f)
    default = np.array([0.0, 1.0, 0.0, 1.0], f)
    filt = np.where(th < f(0.5), default, bb).astype(f)
    x1, y1, x2, y2 = filt[:, 0], filt[:, 1], filt[:, 2], filt[:, 3]

    def resize_side(small, large):
        side = (large - small).astype(f)
        new_side = (side * f(FACTOR)).astype(f)
        center = ((small + large) / f(2)).astype(f)
        half = (new_side / f(2)).astype(f)
        new_min = np.clip((center - half).astype(f), f(0), f(1)).astype(f)
        new_max = np.clip((center + half).astype(f), f(0), f(1)).astype(f)
        return new_min, new_max

    nx1, nx2 = resize_side(x1, x2)
    ny1, ny2 = resize_side(y1, y2)
    # reference: boxes = stack([nx1, ny1, nx2, ny2]); crop uses [y1,x1,y2,x2]
    by1, bx1, by2, bx2 = nx1, ny1, nx2, ny2

    idx = np.arange(CH, dtype=f)
    ys = (by1[:, None] * f(H - 1)).astype(f) + (
        idx[None, :] * (((by2 - by1) * f(H - 1)).astype(f) / f(CH - 1)).astype(f)[:, None]
    ).astype(f)
    ys = ys.astype(f)
    xs = (bx1[:, None] * f(W - 1)).astype(f) + (
        idx[None, :] * (((bx2 - bx1) * f(W - 1)).astype(f) / f(CW - 1)).astype(f)[:, None]
    ).astype(f)
    xs = xs.astype(f)

    ys = np.where((ys >= f(0)) & (ys <= f(H - 1)), ys, BAD).astype(f)
    xs = np.where((xs >= f(0)) & (xs <= f(W - 1)), xs, BAD).astype(f)
    return ys, xs


def _axis_window(cs):
    """Row window [lo, hi] covering floor/ceil of all valid sample coords."""
    v = cs > BAD / 2
    if not v.any():
        return 0, 1
    lo = int(np.floor(cs[v].min()))
    hi = int(np.ceil(cs[v].max()))
    return max(lo, 0), min(hi, H - 1)


def _segments(shifted_list, n_chunks):
    """Per-chunk matmul column segments for a slot.

    shifted_list: per-member shifted coords [224] (BAD entries excluded).
    Returns list of (chunk, a, b, start, stop) with per-column-exact flags.
    """
    masks = []
    for kh in range(n_chunks):
        m = np.zeros(CH, bool)
        for cs in shifted_list:
            valid = cs > BAD / 2
            m |= valid & (cs > 128 * kh - 1 - EPS) & (cs < 128 * kh + 128 + EPS)
        masks.append(m)
    covered = np.zeros(CH, bool)
    for m in masks:
        covered |= m
    masks[0] |= ~covered  # never-covered cols: zero weights, but psum must be written

    segs = []
    written = np.zeros(CH, bool)
    for kh in range(n_chunks):
        m = masks[kh]
        # split runs of m where "previously written" status changes, so each
        # matmul's psum region is uniformly fresh or uniformly accumulated
        # (the interpreter asserts all-or-none pending-zero per matmul)
        key = m.astype(np.int8) * (1 + written.astype(np.int8))
        i = 0
        while i < CH:
            if not m[i]:
                i += 1
                continue
            j = i
            while j < CH and key[j] == key[i]:
                j += 1
            segs.append((kh, i, j))
            i = j
        written |= m
    return segs


def _hat(cs, rows):
    """Hat weights [rows, 224] fp32 for shifted coords cs (BAD -> 0)."""
    rr = np.arange(rows, dtype=np.float32)[:, None]
    w = np.maximum(np.float32(0), np.float32(1) - np.abs(cs[None, :] - rr))
    return w.astype(np.float32)


def _plan(threshold, bboxes, images):
    import ml_dtypes
    bf16 = np.dtype(ml_dtypes.bfloat16)

    ys, xs = _host_coords(threshold, bboxes)
    f = np.float32

    info = []
    for n in range(N_FULL):
        ylo, yhi = _axis_window(ys[n])
        xlo, xhi = _axis_window(xs[n])
        kh = (yhi - ylo) // 128 + 1
        kw = (xhi - xlo) // 128 + 1
        ydir = bool(ys[n, -1] >= ys[n, 0])
        xdir = bool(xs[n, -1] >= xs[n, 0])
        info.append((n, ylo, yhi, xlo, xhi, kh, kw, ydir, xdir))

    order = sorted(
        range(N_FULL),
        key=lambda n: (info[n][5], info[n][6], info[n][7], info[n][8],
                       info[n][2] - info[n][1], info[n][4] - info[n][3]),
    )

    slots = []          # per slot: dict with dims, segments
    in_maps = [dict() for _ in range(N_CORES)]
    perm = np.zeros(N_FULL, np.int64)  # perm[core*PER+slot] = original n

    for k in range(PER_CORE):
        members = [order[k * N_CORES + c] for c in range(N_CORES)]
        KH = max(info[n][5] for n in members)
        KW = max(info[n][6] for n in members)
        ROWS, COLS = 128 * KH, 128 * KW
        ysh, xsh = [], []
        for core, n in enumerate(members):
            _, ylo, yhi, xlo, xhi, *_ = info[n]
            r0 = min(max(ylo, 0), H - ROWS)
            c0 = min(max(xlo, 0), W - COLS)
            ysn = np.where(ys[n] > BAD / 2, ys[n] - f(r0), BAD).astype(f)
            xsn = np.where(xs[n] > BAD / 2, xs[n] - f(c0), BAD).astype(f)
            ysh.append(ysn)
            xsh.append(xsn)
            img = np.ascontiguousarray(
                images[n, r0:r0 + ROWS, c0:c0 + COLS, :]).astype(bf16)
            ry = np.zeros((ROWS, IPAD), f)
            ry[:, :CH] = _hat(ysn, ROWS)
            rx = np.zeros((COLS, IPAD), f)
            rx[:, :CW] = _hat(xsn, COLS)
            in_maps[core][f"img{k}"] = img
            in_maps[core][f"ry{k}"] = np.ascontiguousarray(ry.astype(bf16))
            in_maps[core][f"rx{k}"] = np.ascontiguousarray(rx.astype(bf16))
            perm[core * PER_CORE + k] = n
        slots.append(dict(
            KH=KH, KW=KW,
            ysegs=_segments(ysh, KH),
            xsegs=_segments(xsh, KW),
        ))

    sig = tuple(
        (s["KH"], s["KW"], tuple(s["ysegs"]), tuple(s["xsegs"])) for s in slots
    )
    return slots, in_maps, perm, sig


# ---------------------------------------------------------------- balancing

DMA_NSB = 0.3855  # ns per per-partition byte
# engine copy costs per op, by (engine, 224-wide group count)
CPY = {
    ("gpsimd", 1): 187.0, ("gpsimd", 2): 373.0,
    ("vector", 1): 358.0, ("vector", 2): 592.0,
    ("scalar", 1): 372.0, ("scalar", 2): 558.0,
}


class _Balancer:
    """Greedy load balancer over the shared engine/queue resources."""

    def __init__(self):
        self.load = {"SP": 0.0, "ACT": 0.0, "POOL": 0.0, "DVE": 0.0}

    def pick_dma(self, cost):
        r = min(("SP", "ACT", "POOL"), key=lambda x: self.load[x])
        self.load[r] += cost
        return {"SP": "sync", "ACT": "scalar", "POOL": "gpsimd"}[r]

    def pick_copy(self, ngroups):
        # GPSIMD cannot access PSUM (walrus verifier) - DVE/Act only
        opts = [("DVE", "vector"), ("ACT", "scalar")]
        r, eng = min(opts, key=lambda x: self.load[x[0]] + CPY[(x[1], ngroups)])
        self.load[r] += CPY[(eng, ngroups)]
        return eng


# ---------------------------------------------------------------- device build

def _build_nc(slots):
    from concourse import bacc, tile
    import concourse.mybir as mybir

    dt = mybir.dt
    F32 = dt.float32
    F32R = dt.float32r
    BF16 = dt.bfloat16
    AF = mybir.ActivationFunctionType

    nc = bacc.Bacc(None, target_bir_lowering=False)

    def copy_op(eng, out, in_):
        if eng == "scalar":
            nc.scalar.activation(out, in_, AF.Copy, bias=0.0, scale=1.0)
        else:
            getattr(nc, eng).tensor_copy(out, in_)

    img_d, ry_d, rx_d = [], [], []
    for k, s in enumerate(slots):
        KH, KW = s["KH"], s["KW"]
        img_d.append(nc.declare_dram_parameter(
            f"img{k}", [128 * KH, 128 * KW, C], BF16, isOutput=False))
        ry_d.append(nc.declare_dram_parameter(
            f"ry{k}", [128 * KH, IPAD], BF16, isOutput=False))
        rx_d.append(nc.declare_dram_parameter(
            f"rx{k}", [128 * KW, IPAD], BF16, isOutput=False))
    out_d = nc.declare_dram_parameter(
        "out", [PER_CORE, CH, CW, C], BF16, isOutput=True)

    bal = _Balancer()

    with tile.TileContext(nc) as tc:
        with (
            tc.tile_pool(name="img", bufs=1) as ipool,
            tc.tile_pool(name="wts", bufs=1) as wpool,
            tc.tile_pool(name="vt", bufs=3) as vpool,
            tc.tile_pool(name="osb", bufs=4) as opool,
            tc.tile_pool(name="psv", bufs=5, space="PSUM") as psv_pool,
            tc.tile_pool(name="pso", bufs=3, space="PSUM") as pso_pool,
        ):
            img_t, ry_t, rx_t = [], [], []

            def load_slot(k):
                s = slots[k]
                KH, KW = s["KH"], s["KW"]
                it = ipool.tile([128, KH, 128 * KW, C], BF16,
                                name=f"img_t{k}", tag=f"img{k}")
                q = bal.pick_dma(KH * KW * 128 * C * 2 * DMA_NSB)
                getattr(nc, q).dma_start(
                    out=it[:],
                    in_=img_d[k].rearrange("(kh p) w c -> p kh w c", p=128))
                ryt = wpool.tile([128, KH, IPAD], BF16,
                                 name=f"ry_t{k}", tag=f"ry{k}")
                q = bal.pick_dma(KH * IPAD * 2 * DMA_NSB)
                getattr(nc, q).dma_start(
                    out=ryt[:],
                    in_=ry_d[k].rearrange("(kh p) i -> p kh i", p=128))
                rxt = wpool.tile([128, KW, IPAD], BF16,
                                 name=f"rx_t{k}", tag=f"rx{k}")
                q = bal.pick_dma(KW * IPAD * 2 * DMA_NSB)
                getattr(nc, q).dma_start(
                    out=rxt[:],
                    in_=rx_d[k].rearrange("(kw p) i -> p kw i", p=128))
                img_t.append(it)
                ry_t.append(ryt)
                rx_t.append(rxt)

            PREFETCH = 3
            for k in range(min(PREFETCH, PER_CORE)):
                load_slot(k)

            for k in range(PER_CORE):
                s = slots[k]
                KH, KW = s["KH"], s["KW"]
                if k + PREFETCH < PER_CORE:
                    load_slot(k + PREFETCH)

                # ---- stage 1: chains q=(c,wk) -> vt block q, paired in psum
                vt = vpool.tile([128, 3 * 4 * CH], BF16, name=f"vt{k}", tag="vt")
                chains = [(c, wk) for c in range(C) for wk in range(KW)]
                nseg = len(s["ysegs"])
                for p0 in range(0, len(chains), 2):
                    pair = chains[p0:p0 + 2]
                    pv = psv_pool.tile([128, 448], F32, name=f"pv{k}_{p0}",
                                       tag="psv", padded_shape=[128, 512])
                    total = len(pair) * nseg
                    mi = 0
                    for pos, (c, wk) in enumerate(pair):
                        for (kh, a, b) in s["ysegs"]:
                            nc.tensor.matmul(
                                pv[:, pos * CH + a:pos * CH + b],
                                img_t[k][:, kh, wk * 128:(wk + 1) * 128, c],
                                ry_t[k][:, kh, a:b],
                                start=(mi == 0), stop=(mi == total - 1),
                                skip_group_check=True)
                            mi += 1
                    eng = bal.pick_copy(len(pair))
                    copy_op(eng, vt[:, p0 * CH:(p0 + len(pair)) * CH],
                            pv[:, :len(pair) * CH])

                # ---- stage 2: chains (ic, c); (ic,c0,c1) pairs + c2 pair
                osb = [
                    opool.tile([112, CW, C], BF16, name=f"osb{k}_{ic}", tag=f"osb{ic}")
                    for ic in range(2)
                ]

                nxseg = len(s["xsegs"])

                def chain_mm(po, pos, ic, c, first, last, s=s, KW=KW, vt=vt, k=k):
                    for si, (kw, a, b) in enumerate(s["xsegs"]):
                        dst = po[:, pos, a:b] if len(po.shape) == 3 else po[:, a:b]
                        nc.tensor.matmul(
                            dst,
                            vt[:, (c * KW + kw) * CH + ic * 112:
                               (c * KW + kw) * CH + ic * 112 + 112],
                            rx_t[k][:, kw, a:b],
                            start=(first and si == 0),
                            stop=(last and si == nxseg - 1),
                            skip_group_check=True)

                for ic in range(2):
                    po = pso_pool.tile([112, 2, CW], F32, name=f"po{k}_{ic}",
                                       tag="pso", padded_shape=[112, 2, 256])
                    chain_mm(po, 0, ic, 0, True, False)
                    chain_mm(po, 1, ic, 1, False, True)
                    eng = bal.pick_copy(2)
                    copy_op(eng, osb[ic][:, :, 0:2],
                            po[:].rearrange("p c j -> p j c"))
                po2 = pso_pool.tile([112, 2, CW], F32, name=f"po2_{k}",
                                    tag="pso", padded_shape=[112, 2, 256])
                chain_mm(po2, 0, 0, 2, True, False)
                chain_mm(po2, 1, 1, 2, False, True)
                for ic in range(2):
                    eng = bal.pick_copy(1)
                    copy_op(eng, osb[ic][:, :, 2], po2[:, ic])

                for ic in range(2):
                    q = bal.pick_dma(CW * C * 2 * DMA_NSB)
                    getattr(nc, q).dma_start(
                        out=out_d[k, ic * 112:(ic + 1) * 112], in_=osb[ic][:])

    nc.finalize()
    return nc


def _get_nc(slots, sig):
    if _CACHE.get("sig") != sig:
        _CACHE["nc"] = _build_nc(slots)
        _CACHE["sig"] = sig
    return _CACHE["nc"]


def _ensure_device_platform():
    """If the process pinned jax to cpu (e.g. JAX_PLATFORMS=cpu), re-resolve
    backends so the 8 axon/neuron devices are visible for the PJRT run."""
    import jax
    try:
        if len([d for d in jax.devices() if d.platform != "cpu"]) >= N_CORES:
            return
    except Exception:
        pass
    import os
    os.environ.pop("JAX_PLATFORMS", None)
    try:
        jax.config.update("jax_platforms", None)
    except Exception:
        pass
    for clear in ("clear_backends",):
        try:
            getattr(jax, clear)()
            break
        except Exception:
            pass


def kernel(threshold, bboxes, images):
    from concourse.bass_utils import run_bass_kernel_spmd

    _ensure_device_platform()

    images = np.asarray(images, np.float32)
    slots, in_maps, perm, sig = _plan(threshold, bboxes, images)
    nc = _get_nc(slots, sig)

    res = run_bass_kernel_spmd(nc, in_maps, list(range(N_CORES)))
    _CACHE["last_res"] = res

    out = np.zeros((N_FULL, CH, CW, C), np.float32)
    for core in range(N_CORES):
        o = np.asarray(res.results[core]["out"]).astype(np.float32)
        for k in range(PER_CORE):
            out[perm[core * PER_CORE + k]] = o[k]
    return out
